# revision 1
# baseline (speedup 1.0000x reference)
"""L2 cluster-centroid distance kernel for Trainium2 (8 NeuronCores).

Problem: given embedding [N=1e6, D=128], centers [C=100, D], logits [N, C]:
    assign    = argmax(logits, -1)
    sums      = segment_sum(embedding, assign, C)   # [C, D]
    counts    = segment_sum(1, assign, C)           # [C]
    centroids = sums / max(counts, 1)
    out[c]    = ||centers[c] - centroids[c]||  (0 for empty clusters)

Strategy (data-parallel over N, 8 cores):
  Each core processes ROWS = 124928 rows (976 sub-blocks of 128 rows,
  grouped into 61 chunks of 16 sub-blocks so every DMA is >= 0.8 MiB and
  fully contiguous). Per sub-block the one-hot assignment matrix is built
  on the Vector engine (row-max + is_equal) and the segment sums + counts
  are accumulated on the Tensor engine into PSUM:
      sums_psum   += onehot.T @ emb      (lhsT = onehot [128, 100])
      counts_psum += onehot.T @ ones
  At the end each core DMAs a [C, D+1] partial (sums | counts) to HBM.
  The host adds the 8 partials plus a 576-row tail and does the final
  (tiny) centroid/distance math.
"""

import numpy as np

N = 1_000_000
D = 128
C = 100
N_CORES = 8
P = 128            # rows per sub-block == SBUF partitions == matmul K
T = 16             # sub-blocks per chunk (1 MiB embedding DMA)
CHUNKS = 61        # chunks per core
ROWS = CHUNKS * T * P          # 124928 rows per core
N_DEV = N_CORES * ROWS         # 999424 rows on device; tail handled on host

_CACHE = {}


def _build_bass(rows=ROWS, chunk_ts=None):
    import concourse.bacc as bacc
    import concourse.tile as tile
    from concourse import mybir

    if chunk_ts is None:
        # Small chunks at the start (fast pipeline ramp), big 2 MiB-DMA
        # chunks for bandwidth, small chunks at the end so the compute
        # pipeline drains quickly after the last DMA. Small chunks get
        # their own tile tags (own SBUF slots) so their DMAs can issue
        # early instead of waiting on big-chunk buffer releases.
        chunk_ts = [64] * 15 + [8, 4, 4]
    assert rows == sum(chunk_ts) * P
    tmax = max(chunk_ts)
    nc = bacc.Bacc("TRN2", target_bir_lowering=False, debug=False)
    emb = nc.dram_tensor("embedding", [rows, D], mybir.dt.float32, kind="ExternalInput")
    logit = nc.dram_tensor("logits", [rows, C], mybir.dt.float32, kind="ExternalInput")
    part = nc.dram_tensor("partial", [C, D + 1], mybir.dt.float32, kind="ExternalOutput")

    with tile.TileContext(nc) as tc:
        with (
            tc.tile_pool(name="io", bufs=3) as io_pool,
            tc.tile_pool(name="oh", bufs=3) as oh_pool,
            tc.tile_pool(name="small", bufs=1) as small_pool,
            tc.tile_pool(name="psum", bufs=1, space="PSUM") as psum_pool,
        ):
            ones = small_pool.tile([P, 1], mybir.dt.bfloat16)
            nc.vector.memset(ones, 1.0)
            # One-hot is padded M=100 -> 128 (zero columns) so bf16 matmuls
            # get fast-weight-load (needs NumWeights==128). PSUM rows C:P
            # are garbage-free zeros; host reads rows :C.
            psum_sums = psum_pool.tile([P, D], mybir.dt.float32)
            psum_cnt = psum_pool.tile([P, 1], mybir.dt.float32)

            off = 0
            for k, t in enumerate(chunk_ts):
                # Row r = off + p*t + n: per (k, p) the t rows are
                # contiguous in HBM -> fully contiguous DMA.
                emb_v = emb[off : off + P * t, :].rearrange("(p n) d -> p n d", n=t)
                log_v = logit[off : off + P * t, :].rearrange("(p n) c -> p n c", n=t)
                off += P * t
                # fp32 HBM -> bf16 SBUF cast during DMA (SWDGE/gpsimd only)
                et = io_pool.tile([P, t, D], mybir.dt.bfloat16, tag="emb", padded_shape=[P, tmax, D])
                lt = io_pool.tile([P, t, C], mybir.dt.float32, tag="log", padded_shape=[P, tmax, C])
                nc.sync.dma_start(out=lt, in_=log_v)
                nc.gpsimd.dma_start(out=et, in_=emb_v)

                mx = oh_pool.tile([P, t, 1], mybir.dt.float32, tag="mx", padded_shape=[P, tmax, 1])
                nc.vector.reduce_max(out=mx, in_=lt, axis=mybir.AxisListType.X)
                oh = oh_pool.tile([P, t, P], mybir.dt.bfloat16, tag="oh", padded_shape=[P, tmax, P])
                nc.gpsimd.memset(oh[:, :, C:P], 0.0)
                nc.vector.tensor_tensor(
                    out=oh[:, :, 0:C],
                    in0=lt,
                    in1=mx.to_broadcast([P, t, C]),
                    op=mybir.AluOpType.is_equal,
                )
                for n in range(t):
                    first = (k == 0) and (n == 0)
                    last = (k == len(chunk_ts) - 1) and (n == t - 1)
                    nc.tensor.matmul(
                        out=psum_sums[:, :],
                        lhsT=oh[:, n, :],
                        rhs=et[:, n, :],
                        start=first,
                        stop=last,
                        skip_group_check=True,
                    )
                    nc.tensor.matmul(
                        out=psum_cnt[:, :],
                        lhsT=oh[:, n, :],
                        rhs=ones[:, :],
                        start=first,
                        stop=last,
                        skip_group_check=True,
                    )

            outt = small_pool.tile([C, D + 1], mybir.dt.float32)
            nc.vector.tensor_copy(out=outt[:, 0:D], in_=psum_sums[0:C, :])
            nc.vector.tensor_copy(out=outt[:, D : D + 1], in_=psum_cnt[0:C, :])
            nc.sync.dma_start(out=part[:, :], in_=outt[:, :])

    nc.compile()
    return nc


def _get_nc():
    if "nc" not in _CACHE:
        _CACHE["nc"] = _build_bass()
    return _CACHE["nc"]


def _finalize(sums, counts, centers):
    centroids = sums / np.maximum(counts, 1.0)[:, None]
    delta = centers.astype(np.float64) - centroids
    sq = np.sum(delta * delta, axis=1)
    dist = np.where(sq > 0, np.sqrt(np.where(sq > 0, sq, 1.0)), 0.0)
    return np.where(counts > 0, dist, 0.0).astype(np.float32)


def kernel(embedding, centers, logits):
    from concourse.bass_utils import run_bass_kernel_spmd

    embedding = np.asarray(embedding, dtype=np.float32)
    centers = np.asarray(centers, dtype=np.float32)
    logits = np.asarray(logits, dtype=np.float32)

    nc = _get_nc()
    in_maps = []
    for c in range(N_CORES):
        lo = c * ROWS
        in_maps.append(
            {
                "embedding": np.ascontiguousarray(embedding[lo : lo + ROWS]),
                "logits": np.ascontiguousarray(logits[lo : lo + ROWS]),
            }
        )
    res = run_bass_kernel_spmd(nc, in_maps, core_ids=list(range(N_CORES)))

    sums = np.zeros((C, D), np.float64)
    counts = np.zeros((C,), np.float64)
    for r in res.results:
        p = r["partial"].astype(np.float64)
        sums += p[:, :D]
        counts += p[:, D]

    # Tail rows the device grid doesn't cover (N - N_DEV = 576 rows).
    te = embedding[N_DEV:]
    tl = logits[N_DEV:]
    if te.shape[0]:
        a = np.argmax(tl, axis=1)
        np.add.at(sums, a, te.astype(np.float64))
        np.add.at(counts, a, 1.0)

    return _finalize(sums, counts, centers)

